# revision 2
# baseline (speedup 1.0000x reference)
"""CurvatureEstimator Trainium2 kernel — 8-core data-parallel (batch sharded).

Self-contained: builds constants inline, shards the full input across 8
NeuronCores (2 batch images each), runs one SPMD Bass kernel, gathers output.

Per-core pipeline (all matmuls bf16, PSUM fp32):
  A: fused flip+H-convs: lhsT = X-tile [h128, w128 (c-strided)] (data stationary),
     rhs = BandA [h128, 3*OW] -> psum [w, {u|v|s}] (u=T_H x, v=T2_H x, s=B_H x).
  B: band-stationary W-convs on flipped tiles:
     a=BoxW@u, p=TW@s, den=BoxW@v (+) T2W@s (PSUM-accumulated denominator).
  C: fp32 combine: 2*sqrt(a^2+p^2)*recip(den)  (recip via fast NR approx).
  F: flip back via identity matmuls, re-interleave channels, DMA-cast out.
"""
import sys
if "/opt/trn_rl_repo" not in sys.path:
    sys.path.insert(0, "/opt/trn_rl_repo")
import numpy as np
import concourse.bass as bass
import concourse.tile as tile
from concourse import bacc, mybir
from concourse.bass_utils import run_bass_kernel_spmd

N_CORES = 8
N_TAP = 5

_cache = {}


def _tapval(q, t):
    if abs(t) > N_TAP:
        return 0.0
    return float(t) if q == 0 else (float(t * t) if q == 1 else 1.0)


def block_plan(S):
    ins, outs, widths = [], [], []
    pos = 0
    while pos < S:
        a = 0 if pos == 0 else pos - N_TAP
        if a + 128 >= S:
            a = S - 128
        lo = pos
        hi = S if a + 128 >= S else min(S, a + 128 - N_TAP)
        ins.append(a); outs.append(lo); widths.append(hi - lo)
        pos = hi
    return ins, outs, widths


def make_banda(HS, OS, OW):
    bands = []
    for b in range(len(HS)):
        B = np.zeros((128, 3 * OW[b]), dtype=np.float32)
        for j in range(OW[b]):
            orow = OS[b] + j
            for t in range(-N_TAP, N_TAP + 1):
                k = orow + t - HS[b]
                if 0 <= k < 128:
                    for q in range(3):
                        B[k, q * OW[b] + j] = _tapval(q, t)
        bands.append(B)
    return bands


def make_bandb(WS, WOS, WOW):
    bands = []
    for b in range(len(WS)):
        per_q = []
        for q in range(3):
            B = np.zeros((128, WOW[b]), dtype=np.float32)
            for j in range(WOW[b]):
                ow = WOS[b] + j
                for t in range(-N_TAP, N_TAP + 1):
                    k = ow + t - WS[b]
                    if 0 <= k < 128:
                        B[k, j] = _tapval(q, t)
            per_q.append(B)
        bands.append(per_q)
    return bands


def build_kernel(B_PER_CORE=2, H=512, W=512, C=8):
    HS, OS, OW = block_plan(H)
    WS, WOS, WOW = block_plan(W)
    NHB, NWB = len(HS), len(WS)
    NHC = H // 128
    WC = W * C
    bf16 = mybir.dt.bfloat16
    f32 = mybir.dt.float32

    banda_np = make_banda(HS, OS, OW)
    bandb_np = make_bandb(WS, WOS, WOW)

    nc = bacc.Bacc("TRN2", target_bir_lowering=False, debug=False)
    edges = nc.dram_tensor("edges", [B_PER_CORE, H, W, C], f32, kind="ExternalInput").ap()
    out = nc.dram_tensor("out", [B_PER_CORE, H, W, C], f32, kind="ExternalOutput").ap()
    consts = {}
    banda_d = []
    for b in range(NHB):
        nm = f"banda{b}"
        consts[nm] = banda_np[b]
        banda_d.append(nc.dram_tensor(nm, list(banda_np[b].shape), f32, kind="ExternalInput").ap())
    bandb_d = []
    for b in range(NWB):
        row = []
        for q in range(3):
            nm = f"bandb{b}q{q}"
            consts[nm] = bandb_np[b][q]
            row.append(nc.dram_tensor(nm, list(bandb_np[b][q].shape), f32, kind="ExternalInput").ap())
        bandb_d.append(row)
    consts["ident"] = np.eye(128, dtype=np.float32)
    ident_d = nc.dram_tensor("ident", [128, 128], f32, kind="ExternalInput").ap()

    with tile.TileContext(nc) as tc:
        with (
            tc.tile_pool(name="bandpool", bufs=1) as bandpool,
            tc.tile_pool(name="xtiles", bufs=1) as xpool,
            tc.tile_pool(name="uvs", bufs=2) as uvspool,
            tc.tile_pool(name="outt", bufs=1) as outpool,
            tc.tile_pool(name="stage", bufs=2) as stagepool,
            tc.tile_pool(name="psA", bufs=2, space="PSUM") as psA,
            tc.tile_pool(name="psB", bufs=1, space="PSUM") as psB,
            tc.tile_pool(name="psF", bufs=1, space="PSUM") as psF,
            tc.tile_pool(name="cscr", bufs=2) as cpool,
        ):
            banda_t = []
            for b in range(NHB):
                t = bandpool.tile([128, 3 * OW[b]], bf16, tag=f"banda{b}")
                nc.gpsimd.dma_start(t[:], banda_d[b][:])
                banda_t.append(t)
            bandb_t = []
            for b in range(NWB):
                row = []
                for q in range(3):
                    t = bandpool.tile([128, WOW[b]], bf16, tag=f"bandb{b}{q}")
                    nc.gpsimd.dma_start(t[:], bandb_d[b][q][:])
                    row.append(t)
                bandb_t.append(row)
            ident_t = bandpool.tile([128, 128], bf16, tag="ident")
            nc.gpsimd.dma_start(ident_t[:], ident_d[:])

            for img in range(B_PER_CORE):
                xts = []
                for b in range(NHB):
                    xt = xpool.tile([128, WC], bf16, tag=f"x{b}")
                    nc.gpsimd.dma_start(
                        xt[:], edges[img, HS[b]:HS[b] + 128].rearrange("h w c -> h (w c)"))
                    xts.append(xt)

                out_tiles = {}
                for c in range(C):
                    uvs = []
                    for wb in range(NWB):
                        uvs.append(uvspool.tile([128, 3 * H], bf16, tag=f"uvs{wb}", name=f"uvs{wb}"))
                    for hb in range(NHB):
                        xv = xts[hb][:].rearrange("h (w c) -> h w c", c=C)
                        for wb in range(NWB):
                            lhsT = xv[:, WS[wb]:WS[wb] + 128, c]
                            pa = psA.tile([128, 3 * OW[hb]], f32, tag="psA")
                            nc.tensor.matmul(pa[:], lhsT, banda_t[hb][:], start=True, stop=True)
                            src = pa[:].rearrange("p (q j) -> p q j", q=3)
                            dst = uvs[wb][:].rearrange("p (q h) -> p q h", q=3)[:, :, OS[hb]:OS[hb] + OW[hb]]
                            if hb % 2 == 0:
                                nc.vector.tensor_copy(dst, src)
                            else:
                                nc.scalar.copy(dst, src)
                    for wb in range(NWB):
                        wN = WOW[wb]
                        uview = uvs[wb][:, 0:H]
                        vview = uvs[wb][:, H:2 * H]
                        sview = uvs[wb][:, 2 * H:3 * H]
                        pa_ = psB.tile([128, H], f32, tag="psBa")
                        pp_ = psB.tile([128, H], f32, tag="psBp")
                        pd_ = psB.tile([128, H], f32, tag="psBd")
                        nc.tensor.matmul(pa_[0:wN, :], bandb_t[wb][2][:, 0:wN], uview, start=True, stop=True)
                        nc.tensor.matmul(pp_[0:wN, :], bandb_t[wb][0][:, 0:wN], sview, start=True, stop=True)
                        nc.tensor.matmul(pd_[0:wN, :], bandb_t[wb][2][:, 0:wN], vview, start=True, stop=False)
                        nc.tensor.matmul(pd_[0:wN, :], bandb_t[wb][1][:, 0:wN], sview, start=False, stop=True)
                        A2 = cpool.tile([128, H], f32, tag="A2")
                        P2 = cpool.tile([128, H], f32, tag="P2")
                        N2 = cpool.tile([128, H], f32, tag="N2")
                        NM = cpool.tile([128, H], f32, tag="NM")
                        R = cpool.tile([128, H], f32, tag="R")
                        nc.scalar.square(A2[0:wN, :], pa_[0:wN, :])
                        nc.scalar.square(P2[0:wN, :], pp_[0:wN, :])
                        nc.vector.tensor_add(N2[0:wN, :], A2[0:wN, :], P2[0:wN, :])
                        nc.scalar.activation(NM[0:wN, :], N2[0:wN, :],
                                             mybir.ActivationFunctionType.Sqrt, scale=4.0)
                        nc.vector.reciprocal_approx_fast(out=R[0:wN, :], in_=pd_[0:wN, :])
                        ot = outpool.tile([128, H], bf16, tag=f"out{c}_{wb}")
                        nc.vector.tensor_mul(ot[0:wN, :], NM[0:wN, :], R[0:wN, :])
                        out_tiles[(c, wb)] = ot

                for hc in range(NHC):
                    stg = stagepool.tile([128, WC], bf16, tag="stg")
                    for wb in range(NWB):
                        wN = WOW[wb]
                        for half in range(2):
                            pf = psF.tile([128, 512], f32, tag=f"psF{half}")
                            pfv = pf[:].rearrange("p (j c) -> p j c", c=4)
                            for ci in range(4):
                                c = half * 4 + ci
                                lhsT = out_tiles[(c, wb)][0:wN, hc * 128:(hc + 1) * 128]
                                nc.tensor.matmul(pfv[:, 0:wN, ci], lhsT, ident_t[0:wN, 0:wN],
                                                 start=True, stop=True)
                            dst = stg[:].rearrange("p (w c) -> p w c", c=C)[
                                :, WOS[wb]:WOS[wb] + wN, half * 4:half * 4 + 4]
                            src = pf[:, 0:4 * wN].rearrange("p (j c) -> p j c", c=4)
                            if wb % 2 == 0:
                                nc.vector.tensor_copy(dst, src)
                            else:
                                nc.scalar.copy(dst, src)
                    nc.gpsimd.dma_start(
                        out[img, hc * 128:(hc + 1) * 128].rearrange("h w c -> h (w c)"), stg[:])
    nc.compile()
    return nc, consts


def _get_kernel(bpc, H, W, C):
    key = (bpc, H, W, C)
    if key not in _cache:
        _cache[key] = build_kernel(bpc, H, W, C)
    return _cache[key]


def kernel(edges: np.ndarray) -> np.ndarray:
    edges = np.ascontiguousarray(edges, dtype=np.float32)
    B, H, W, C = edges.shape
    assert B % N_CORES == 0
    bpc = B // N_CORES
    nc, consts = _get_kernel(bpc, H, W, C)
    in_maps = []
    for i in range(N_CORES):
        m = {"edges": edges[i * bpc:(i + 1) * bpc]}
        m.update(consts)
        in_maps.append(m)
    res = run_bass_kernel_spmd(nc, in_maps, list(range(N_CORES)))
    return np.concatenate([res.results[i]["out"] for i in range(N_CORES)], axis=0)
